# revision 16
# baseline (speedup 1.0000x reference)
"""DeepSeek/Llama-style transformer block on 8 Trainium2 NeuronCores.

Sharding: batch x sequence (2 batches x 4 query-chunks of 512 tokens), fully
data-parallel, no collectives. Each core computes K/V for its batch's full
sequence (small replicated work) and everything else only for its own 512
tokens. Key order is rotated per-core so each core's own tokens sit at
columns 0:512 of its rolled sequence (softmax is invariant to key order as
long as the mask columns are rotated identically).

On-chip layout is feature-major ([feature, token]) throughout; the host
pre-transposes x and all weights so every DMA is contiguous, and folds the
rmsnorm weights + 1/sqrt(hd) into the projection weights. The per-token
rmsnorm scale commutes through the projections and is applied to the
projection outputs (Q/K/V, gate/up) instead of the inputs.
"""

import numpy as np
import ml_dtypes

import concourse.bass as bass
import concourse.tile as tile
import concourse.bacc as bacc
from concourse import mybir
from concourse.bass_utils import run_bass_kernel_spmd

H = 2048
S = 2048
T = 512          # own tokens per core
HD = 128
NH = 16
NKV = 4
KV = NKV * HD    # 512
FF = 8192
P = 128
ET = H // P      # 16
FT = FF // P     # 64
EPS = 1e-5
REP = NH // NKV


def _build():
    nc = bacc.Bacc(None, target_bir_lowering=False)
    f32 = mybir.dt.float32
    f32r = mybir.dt.float32r
    bf16 = mybir.dt.bfloat16

    xT = nc.dram_tensor("xT", [H, S], f32r, kind="ExternalInput")
    maskT = nc.dram_tensor("maskT", [S, T], bf16, kind="ExternalInput")
    wqT = nc.dram_tensor("wqT", [H, H], f32r, kind="ExternalInput")
    wkT = nc.dram_tensor("wkT", [H, KV], f32r, kind="ExternalInput")
    wvT = nc.dram_tensor("wvT", [H, KV], f32r, kind="ExternalInput")
    woT = nc.dram_tensor("woT", [H, H], f32r, kind="ExternalInput")
    wgT = nc.dram_tensor("wgT", [H, FF], f32r, kind="ExternalInput")
    wuT = nc.dram_tensor("wuT", [H, FF], f32r, kind="ExternalInput")
    wdT = nc.dram_tensor("wdT", [FF, H], f32r, kind="ExternalInput")
    ones_d = nc.dram_tensor("ones_d", [P, 1], f32r, kind="ExternalInput")
    outT = nc.dram_tensor("outT", [H, T], f32, kind="ExternalOutput")

    # DRAM scratch
    qT_d = nc.dram_tensor("qT_d", [H, T], f32r)
    kT_d = nc.dram_tensor("kT_d", [KV, S], f32r)
    v_d = nc.dram_tensor("v_d", [S, KV], f32r)
    act_d = nc.dram_tensor("act_d", [FF, T], f32r)
    row_d = nc.dram_tensor("row_d", [1, S], f32)

    xT_t = xT.rearrange("(e p) s -> e p s", p=P)
    wqT_t = wqT.rearrange("(e p) d -> e p d", p=P)
    wkT_t = wkT.rearrange("(e p) d -> e p d", p=P)
    wvT_t = wvT.rearrange("(e p) d -> e p d", p=P)
    woT_t = woT.rearrange("(e p) d -> e p d", p=P)
    wgT_t = wgT.rearrange("(e p) f -> e p f", p=P)
    wuT_t = wuT.rearrange("(e p) f -> e p f", p=P)
    wdT_t = wdT.rearrange("(f p) o -> f p o", p=P)
    qT_d_t = qT_d.rearrange("(g p) t -> g p t", p=P)
    kT_d_t = kT_d.rearrange("(g p) s -> g p s", p=P)
    v_d_t = v_d.rearrange("(g p) d -> g p d", p=P)
    act_d_t = act_d.rearrange("(f p) t -> f p t", p=P)
    maskT_t = maskT.rearrange("(k p) t -> k p t", p=P)
    outT_t = outT.rearrange("(o p) t -> o p t", p=P)

    from contextlib import ExitStack

    with tile.TileContext(nc) as tc, ExitStack() as top:
        consts = top.enter_context(tc.tile_pool(name="consts", bufs=1))
        psum = top.enter_context(tc.tile_pool(name="psum", bufs=8, space="PSUM"))

        ones = consts.tile([P, 1], f32r)
        nc.sync.dma_start(ones[:], ones_d[:])
        eps_t = consts.tile([P, 1], f32)
        nc.vector.memset(eps_t[:], EPS)
        inv_b = consts.tile([P, S], f32)
        inv_colT = consts.tile([P, ET], f32)
        inv2_b = consts.tile([P, T], f32)

        # ---------------- Phase 0+1: rmsnorm stats + Q/K/V projections -----
        with tc.tile_pool(name="phA", bufs=1) as phA, \
             tc.tile_pool(name="phA2", bufs=2) as phA2, \
             tc.tile_pool(name="phA3", bufs=3) as phA3:
            x_sb = phA.tile([P, ET, S], f32r)
            for e in range(ET):
                nc.sync.dma_start(x_sb[:, e, :], xT_t[e])

            # sum over features of x^2, via ACT square + ones-matmul
            rms_ps = [psum.tile([1, 512], f32, tag="ps", name=f"rms_ps{i}") for i in range(4)]
            for e in range(ET):
                sq = phA2.tile([P, S], f32r, tag="sq")
                nc.scalar.square(sq[:], x_sb[:, e, :])
                for c in range(4):
                    nc.tensor.matmul(
                        rms_ps[c][:], ones[:], sq[:, c * 512:(c + 1) * 512],
                        start=(e == 0), stop=(e == ET - 1))
            inv_row = phA2.tile([1, S], f32, tag="sq")
            for c in range(4):
                nc.scalar.activation(
                    inv_row[:, c * 512:(c + 1) * 512], rms_ps[c][:],
                    mybir.ActivationFunctionType.Sqrt, bias=eps_t[0:1, :],
                    scale=1.0 / H)
            nc.vector.reciprocal(inv_row[:], inv_row[:])
            nc.gpsimd.partition_broadcast(inv_b[:], inv_row[:])
            nc.sync.dma_start(row_d[:], inv_row[:])
            nc.sync.dma_start(inv_colT[:], row_d[0, :].rearrange("(t p) -> p t", p=P))

            # Q^T [H, T]: lhsT = wqT tile, rhs = xT own columns
            for dg in range(4):
                q_ps = [psum.tile([P, 512], f32, tag="ps", name=f"q_ps{dg}_{i}") for i in range(4)]
                for e in range(ET):
                    ws = phA3.tile([P, 512], f32r, tag="wstream")
                    nc.sync.dma_start(ws[:], wqT_t[e][:, dg * 512:(dg + 1) * 512])
                    for dt in range(4):
                        nc.tensor.matmul(
                            q_ps[dt][:], ws[:, dt * 128:(dt + 1) * 128],
                            x_sb[:, e, 0:T], start=(e == 0), stop=(e == ET - 1))
                for dt in range(4):
                    st = phA3.tile([P, 512], f32r, tag="qstage")
                    nc.vector.tensor_mul(st[:], q_ps[dt][:], inv_b[:, 0:T])
                    nc.sync.dma_start(qT_d_t[dg * 4 + dt], st[:])

            # K^T [KV, S]
            for dt in range(4):
                k_ps = [psum.tile([P, 512], f32, tag="ps", name=f"k_ps{dt}_{i}") for i in range(4)]
                for e in range(ET):
                    ws = phA3.tile([P, 128], f32r, tag="wkstream")
                    nc.sync.dma_start(ws[:], wkT_t[e][:, dt * 128:(dt + 1) * 128])
                    for kc in range(4):
                        nc.tensor.matmul(
                            k_ps[kc][:], ws[:],
                            x_sb[:, e, kc * 512:(kc + 1) * 512],
                            start=(e == 0), stop=(e == ET - 1))
                for kc in range(4):
                    st = phA3.tile([P, 512], f32r, tag="qstage")
                    nc.vector.tensor_mul(st[:], k_ps[kc][:],
                                         inv_b[:, kc * 512:(kc + 1) * 512])
                    nc.sync.dma_start(kT_d_t[dt][:, kc * 512:(kc + 1) * 512], st[:])

            # V [S, KV] token-major: lhsT = xT tile (tokens as M), rhs = wvT
            for tg in range(4):
                v_ps = [psum.tile([P, 512], f32, tag="ps", name=f"v_ps{tg}_{i}") for i in range(4)]
                for e in range(ET):
                    ws = phA3.tile([P, 512], f32r, tag="wstream")
                    nc.sync.dma_start(ws[:], wvT_t[e])
                    for tt in range(4):
                        tok = (tg * 4 + tt) * 128
                        nc.tensor.matmul(
                            v_ps[tt][:], x_sb[:, e, tok:tok + 128], ws[:],
                            start=(e == 0), stop=(e == ET - 1))
                for tt in range(4):
                    st = phA3.tile([P, 512], f32r, tag="qstage")
                    nc.scalar.activation(
                        st[:], v_ps[tt][:], mybir.ActivationFunctionType.Copy,
                        scale=inv_colT[:, tg * 4 + tt:tg * 4 + tt + 1])
                    nc.sync.dma_start(v_d_t[tg * 4 + tt], st[:])

        # ---------------- Phase 2: attention ------------------------------
        attn_pool = top.enter_context(tc.tile_pool(name="attnp", bufs=1))
        attnT = attn_pool.tile([P, ET, T], f32r)
        with tc.tile_pool(name="phB", bufs=1) as phB, \
             tc.tile_pool(name="phB2", bufs=2) as phB2, \
             tc.tile_pool(name="phBpt", bufs=ET + 2) as phBpt:
            q_sb = phB.tile([P, ET, T], f32r)
            m_sb = phB.tile([P, ET, T], bf16)
            for e in range(ET):
                nc.sync.dma_start(q_sb[:, e, :], qT_d_t[e])
                nc.sync.dma_start(m_sb[:, e, :], maskT_t[e])

            for g in range(NKV):
                k_g = phB2.tile([P, S], f32r, tag="kg", name=f"k_g{g}")
                nc.sync.dma_start(k_g[:], kT_d_t[g])
                v_g = phB2.tile([P, ET, HD], f32r, tag="vg", name=f"v_g{g}")
                for e in range(ET):
                    nc.sync.dma_start(v_g[:, e, :],
                                      v_d_t[e][:, g * 128:(g + 1) * 128])
                for hh in range(REP):
                    h = g * REP + hh
                    pts = []
                    for kt in range(ET):
                        s_ps = psum.tile([P, 512], f32, tag="ps",
                                         name=f"s_ps{h}_{kt}")
                        nc.tensor.matmul(
                            s_ps[:], k_g[:, kt * 128:(kt + 1) * 128],
                            q_sb[:, h, :], start=True, stop=True)
                        pt = phBpt.tile([P, T], f32r, tag="pt",
                                        name=f"pt{h}_{kt}")
                        nc.vector.tensor_tensor(
                            out=pt[:], in0=s_ps[:], in1=m_sb[:, kt, :],
                            op=mybir.AluOpType.add)
                        nc.scalar.activation(pt[:], pt[:],
                                             mybir.ActivationFunctionType.Exp)
                        pts.append(pt)
                    sum_ps = psum.tile([1, 512], f32, tag="ps",
                                       name=f"sum_ps{h}")
                    for kt in range(ET):
                        nc.tensor.matmul(sum_ps[:], ones[:], pts[kt][:],
                                         start=(kt == 0), stop=(kt == ET - 1))
                    pv_ps = psum.tile([P, 512], f32, tag="ps",
                                      name=f"pv_ps{h}")
                    for kt in range(ET):
                        nc.tensor.matmul(
                            pv_ps[:], v_g[:, kt, :],
                            pts[kt][:], start=(kt == 0), stop=(kt == ET - 1))
                    inv_sum = phB2.tile([1, 512], f32, tag="invs",
                                        name=f"inv_sum{h}")
                    nc.vector.reciprocal(inv_sum[:], sum_ps[:])
                    invS_b = phB2.tile([P, 512], f32, tag="invsb",
                                       name=f"invS_b{h}")
                    nc.gpsimd.partition_broadcast(invS_b[:], inv_sum[:])
                    nc.vector.tensor_mul(attnT[:, h, :], pv_ps[:], invS_b[:])

        # ---------------- Phase 3: o_proj + residual ----------------------
        hT_pool = top.enter_context(tc.tile_pool(name="hTp", bufs=1))
        hT = hT_pool.tile([P, ET, T], f32r)
        with tc.tile_pool(name="phC", bufs=1) as phC, \
             tc.tile_pool(name="phC3", bufs=3) as phC3:
            xo_sb = phC.tile([P, ET, T], f32r)
            for e in range(ET):
                nc.sync.dma_start(xo_sb[:, e, :], xT_t[e][:, 0:T])
            for og in range(4):
                o_ps = [psum.tile([P, 512], f32, tag="ps", name=f"o_ps{og}_{i}") for i in range(4)]
                for e in range(ET):
                    ws = phC3.tile([P, 512], f32r, tag="wostream")
                    nc.sync.dma_start(ws[:], woT_t[e][:, og * 512:(og + 1) * 512])
                    for ot in range(4):
                        nc.tensor.matmul(
                            o_ps[ot][:], ws[:, ot * 128:(ot + 1) * 128],
                            attnT[:, e, :], start=(e == 0), stop=(e == ET - 1))
                for ot in range(4):
                    nc.vector.tensor_tensor(
                        out=hT[:, og * 4 + ot, :], in0=o_ps[ot][:],
                        in1=xo_sb[:, og * 4 + ot, :], op=mybir.AluOpType.add)

        # ---------------- Phase 4: rmsnorm 2 ------------------------------
        with tc.tile_pool(name="phD", bufs=2) as phD:
            rms2_ps = psum.tile([1, 512], f32, tag="ps")
            for o in range(ET):
                sq = phD.tile([P, T], f32r, tag="sq2")
                nc.scalar.square(sq[:], hT[:, o, :])
                nc.tensor.matmul(rms2_ps[:], ones[:], sq[:],
                                 start=(o == 0), stop=(o == ET - 1))
            inv2_row = phD.tile([1, T], f32, tag="i2r")
            nc.scalar.activation(inv2_row[:], rms2_ps[:],
                                 mybir.ActivationFunctionType.Sqrt,
                                 bias=eps_t[0:1, :], scale=1.0 / H)
            nc.vector.reciprocal(inv2_row[:], inv2_row[:])
            nc.gpsimd.partition_broadcast(inv2_b[:], inv2_row[:])

        # ---------------- Phase 5: MLP ------------------------------------
        with tc.tile_pool(name="phE", bufs=1) as phE, \
             tc.tile_pool(name="phE3", bufs=3) as phE3:
            gact = phE.tile([P, 8, T], f32)
            for fg in range(8):
                g_ps = [psum.tile([P, 512], f32, tag="ps", name=f"g_ps{fg}_{i}") for i in range(8)]
                for e in range(ET):
                    ws = phE3.tile([P, 1024], f32r, tag="wgstream")
                    nc.sync.dma_start(ws[:], wgT_t[e][:, fg * 1024:(fg + 1) * 1024])
                    for ft in range(8):
                        nc.tensor.matmul(
                            g_ps[ft][:], ws[:, ft * 128:(ft + 1) * 128],
                            hT[:, e, :], start=(e == 0), stop=(e == ET - 1))
                for ft in range(8):
                    nc.vector.tensor_mul(gact[:, ft, :], g_ps[ft][:], inv2_b[:])
                    nc.scalar.activation(gact[:, ft, :], gact[:, ft, :],
                                         mybir.ActivationFunctionType.Silu)
                u_ps = [psum.tile([P, 512], f32, tag="ps", name=f"u_ps{fg}_{i}") for i in range(8)]
                for e in range(ET):
                    ws = phE3.tile([P, 1024], f32r, tag="wgstream")
                    nc.sync.dma_start(ws[:], wuT_t[e][:, fg * 1024:(fg + 1) * 1024])
                    for ft in range(8):
                        nc.tensor.matmul(
                            u_ps[ft][:], ws[:, ft * 128:(ft + 1) * 128],
                            hT[:, e, :], start=(e == 0), stop=(e == ET - 1))
                for ft in range(8):
                    st = phE3.tile([P, T], f32r, tag="actstage")
                    nc.vector.tensor_mul(st[:], u_ps[ft][:], inv2_b[:])
                    nc.vector.tensor_mul(st[:], st[:], gact[:, ft, :])
                    nc.sync.dma_start(act_d_t[fg * 8 + ft], st[:])

            # down proj + residual
            for og in range(2):
                d_ps = [psum.tile([P, 512], f32, tag="ps", name=f"d_ps{og}_{i}") for i in range(8)]
                for f in range(FT):
                    a_s = phE3.tile([P, T], f32r, tag="actin")
                    nc.sync.dma_start(a_s[:], act_d_t[f])
                    ws = phE3.tile([P, 1024], f32r, tag="wdstream")
                    nc.sync.dma_start(ws[:], wdT_t[f][:, og * 1024:(og + 1) * 1024])
                    for ot in range(8):
                        nc.tensor.matmul(
                            d_ps[ot][:], ws[:, ot * 128:(ot + 1) * 128],
                            a_s[:], start=(f == 0), stop=(f == FT - 1))
                for ot in range(8):
                    st = phE3.tile([P, T], f32, tag="ostage")
                    nc.vector.tensor_tensor(
                        out=st[:], in0=d_ps[ot][:], in1=hT[:, og * 8 + ot, :],
                        op=mybir.AluOpType.add)
                    nc.sync.dma_start(outT_t[og * 8 + ot], st[:])

    nc.finalize()
    return nc


_NC = None
LAST_RESULT = None


def prepare_in_maps(x, attention_mask, wq, wk, wv, wo, norm1_w, norm2_w,
                    w_gate, w_up, w_down):
    x = np.asarray(x, dtype=np.float32)
    mask = np.asarray(attention_mask, dtype=np.float32)[0, 0]  # [S, S]
    n1 = np.asarray(norm1_w, dtype=np.float32)
    n2 = np.asarray(norm2_w, dtype=np.float32)

    wqT = np.ascontiguousarray(
        (np.asarray(wq, np.float32) * (n1[None, :] / np.sqrt(HD))).T)
    wkT = np.ascontiguousarray((np.asarray(wk, np.float32) * n1[None, :]).T)
    wvT = np.ascontiguousarray((np.asarray(wv, np.float32) * n1[None, :]).T)
    woT = np.ascontiguousarray(np.asarray(wo, np.float32).T)
    wgT = np.ascontiguousarray((np.asarray(w_gate, np.float32) * n2[None, :]).T)
    wuT = np.ascontiguousarray((np.asarray(w_up, np.float32) * n2[None, :]).T)
    wdT = np.ascontiguousarray(np.asarray(w_down, np.float32).T)
    ones = np.ones((P, 1), dtype=np.float32)

    in_maps = []
    for c in range(8):
        b, j = divmod(c, 4)
        xr = np.roll(x[b], -T * j, axis=0)            # rolled sequence
        xT = np.ascontiguousarray(xr.T)               # [H, S]
        mrows = mask[T * j:T * (j + 1), :]            # own query rows
        maskT = np.ascontiguousarray(
            np.roll(mrows, -T * j, axis=1).T).astype(ml_dtypes.bfloat16)
        in_maps.append({
            "xT": xT, "maskT": maskT, "wqT": wqT, "wkT": wkT, "wvT": wvT,
            "woT": woT, "wgT": wgT, "wuT": wuT, "wdT": wdT, "ones_d": ones,
        })
    return in_maps


def kernel(x, attention_mask, wq, wk, wv, wo, norm1_w, norm2_w,
           w_gate, w_up, w_down):
    global _NC
    if _NC is None:
        _NC = _build()
    nc = _NC

    in_maps = prepare_in_maps(x, attention_mask, wq, wk, wv, wo, norm1_w,
                              norm2_w, w_gate, w_up, w_down)

    res = run_bass_kernel_spmd(nc, in_maps, core_ids=list(range(8)))
    global LAST_RESULT
    LAST_RESULT = res

    out = np.empty((2, S, H), dtype=np.float32)
    for c in range(8):
        b, j = divmod(c, 4)
        out[b, T * j:T * (j + 1), :] = res.results[c]["outT"].T
    return out


# revision 18
# speedup vs baseline: 1.1742x; 1.1742x over previous
"""DeepSeek/Llama-style transformer block on 8 Trainium2 NeuronCores.

Sharding: batch x sequence (2 batches x 4 query-chunks of 512 tokens), fully
data-parallel, no collectives. Each core computes K/V for its batch's full
sequence (small replicated work) and everything else only for its own 512
tokens. Key order is rotated per-core so each core's own tokens sit at
columns 0:512 of its rolled sequence (softmax is invariant to key order as
long as the mask columns are rotated identically).

On-chip layout is feature-major ([feature, token]) throughout; the host
pre-transposes x and all weights so every DMA is contiguous, and folds the
rmsnorm weights + 1/sqrt(hd) into the projection weights. The per-token
rmsnorm scale commutes through the projections and is applied to the
projection outputs (Q/K/V, gate/up) instead of the inputs. The additive
attention mask is applied multiplicatively as exp(mask) on the exp'd
scores (exact identity), which keeps the mask work on the DVE 2x bf16 path.
Matmuls run in f32r (tf32-like) except the MLP, which runs in bf16 to halve
its weight-streaming DMA (the kernel is at the DMA/PE ridge there).
"""

import numpy as np
import ml_dtypes
from contextlib import ExitStack

import concourse.bass as bass
import concourse.tile as tile
import concourse.bacc as bacc
from concourse import mybir
from concourse.bass_utils import run_bass_kernel_spmd

H = 2048
S = 2048
T = 512          # own tokens per core
HD = 128
NH = 16
NKV = 4
KV = NKV * HD    # 512
FF = 8192
P = 128
ET = H // P      # 16
FT = FF // P     # 64
EPS = 1e-5
REP = NH // NKV


def _build():
    nc = bacc.Bacc(None, target_bir_lowering=False)
    f32 = mybir.dt.float32
    f32r = mybir.dt.float32r
    bf16 = mybir.dt.bfloat16
    Exp = mybir.ActivationFunctionType.Exp
    Sqrt = mybir.ActivationFunctionType.Sqrt
    Silu = mybir.ActivationFunctionType.Silu
    Copy = mybir.ActivationFunctionType.Copy
    ADD = mybir.AluOpType.add

    xT = nc.dram_tensor("xT", [H, S], f32r, kind="ExternalInput")
    maskE = nc.dram_tensor("maskE", [S, T], bf16, kind="ExternalInput")  # exp(mask)^T
    wqT = nc.dram_tensor("wqT", [H, H], f32r, kind="ExternalInput")
    wkT = nc.dram_tensor("wkT", [H, KV], f32r, kind="ExternalInput")
    wvT = nc.dram_tensor("wvT", [H, KV], f32r, kind="ExternalInput")
    woT = nc.dram_tensor("woT", [H, H], f32r, kind="ExternalInput")
    wgT = nc.dram_tensor("wgT", [H, FF], bf16, kind="ExternalInput")
    wuT = nc.dram_tensor("wuT", [H, FF], bf16, kind="ExternalInput")
    wdT = nc.dram_tensor("wdT", [FF, H], bf16, kind="ExternalInput")
    ones_d = nc.dram_tensor("ones_d", [P, 1], f32r, kind="ExternalInput")
    onesb_d = nc.dram_tensor("onesb_d", [P, 1], bf16, kind="ExternalInput")
    outT = nc.dram_tensor("outT", [H, T], f32, kind="ExternalOutput")

    # DRAM scratch
    qT_d = nc.dram_tensor("qT_d", [H, T], f32r)
    kT_d = nc.dram_tensor("kT_d", [KV, S], f32r)
    v_d = nc.dram_tensor("v_d", [S, KV], bf16)
    row_d = nc.dram_tensor("row_d", [1, S], f32)

    xT_t = xT.rearrange("(e p) s -> e p s", p=P)
    wqT_t = wqT.rearrange("(e p) d -> e p d", p=P)
    wkT_t = wkT.rearrange("(e p) d -> e p d", p=P)
    wvT_t = wvT.rearrange("(e p) d -> e p d", p=P)
    woT_t = woT.rearrange("(e p) d -> e p d", p=P)
    wgT_t = wgT.rearrange("(e p) f -> e p f", p=P)
    wuT_t = wuT.rearrange("(e p) f -> e p f", p=P)
    wdT_t = wdT.rearrange("(f p) o -> f p o", p=P)
    qT_d_t = qT_d.rearrange("(g p) t -> g p t", p=P)
    kT_d_t = kT_d.rearrange("(g p) s -> g p s", p=P)
    v_d_t = v_d.rearrange("(g p) d -> g p d", p=P)
    maskE_t = maskE.rearrange("(k p) t -> k p t", p=P)
    outT_t = outT.rearrange("(o p) t -> o p t", p=P)

    with tile.TileContext(nc) as tc, ExitStack() as top:
        consts = top.enter_context(tc.tile_pool(name="consts", bufs=1))
        psum = top.enter_context(tc.tile_pool(name="psum", bufs=8, space="PSUM"))

        ones = consts.tile([P, 1], f32r)
        nc.sync.dma_start(ones[:], ones_d[:])
        ones_bf = consts.tile([P, 1], bf16)
        nc.sync.dma_start(ones_bf[:], onesb_d[:])
        eps_t = consts.tile([P, 1], f32)
        nc.vector.memset(eps_t[:], EPS)
        inv_b = consts.tile([P, S], f32)
        inv_colT = consts.tile([P, ET], f32)
        inv2_b = consts.tile([P, T], f32)

        # ---------------- Phase 0+1: rmsnorm stats + Q/K/V projections -----
        with tc.tile_pool(name="phA", bufs=1) as phA, \
             tc.tile_pool(name="phA2", bufs=2) as phA2, \
             tc.tile_pool(name="phAw", bufs=10) as phAw, \
             tc.tile_pool(name="phA3", bufs=3) as phA3:
            x_sb = phA.tile([P, ET, S], f32r)
            for e in range(ET):
                nc.sync.dma_start(x_sb[:, e, :], xT_t[e])

            # interleaved: rms x^2 sums + Q projection group dg=0
            rms_ps = [psum.tile([1, 512], f32, tag="ps", name=f"rms_ps{i}")
                      for i in range(4)]
            q_ps = [psum.tile([P, 512], f32, tag="ps", name=f"q_ps0_{i}")
                    for i in range(4)]
            for e in range(ET):
                sq = phA2.tile([P, S], f32r, tag="sq", name=f"sq{e}")
                nc.scalar.square(sq[:], x_sb[:, e, :])
                for c in range(4):
                    nc.tensor.matmul(
                        rms_ps[c][:], ones[:], sq[:, c * 512:(c + 1) * 512],
                        start=(e == 0), stop=(e == ET - 1))
                ws = phAw.tile([P, 512], f32r, tag="wstream", name=f"wsq0_{e}")
                nc.sync.dma_start(ws[:], wqT_t[e][:, 0:512])
                for dt in range(4):
                    nc.tensor.matmul(
                        q_ps[dt][:], ws[:, dt * 128:(dt + 1) * 128],
                        x_sb[:, e, 0:T], start=(e == 0), stop=(e == ET - 1))

            inv_row = phA2.tile([1, S], f32, tag="sq")
            for c in range(4):
                nc.scalar.activation(
                    inv_row[:, c * 512:(c + 1) * 512], rms_ps[c][:],
                    Sqrt, bias=eps_t[0:1, :], scale=1.0 / H)
            nc.vector.reciprocal(inv_row[:], inv_row[:])
            nc.gpsimd.partition_broadcast(inv_b[:], inv_row[:])
            nc.sync.dma_start(row_d[:], inv_row[:])
            nc.sync.dma_start(inv_colT[:], row_d[0, :].rearrange("(t p) -> p t", p=P))

            for dt in range(4):
                st = phA3.tile([P, 512], f32r, tag="qstage", name=f"q0st{dt}")
                nc.vector.tensor_mul(st[:], q_ps[dt][:], inv_b[:, 0:T])
                nc.sync.dma_start(qT_d_t[dt], st[:])

            # Q groups dg=1..3
            for dg in range(1, 4):
                q_psl = [psum.tile([P, 512], f32, tag="ps", name=f"q_ps{dg}_{i}")
                         for i in range(4)]
                for e in range(ET):
                    ws = phAw.tile([P, 512], f32r, tag="wstream",
                                   name=f"wsq{dg}_{e}")
                    nc.sync.dma_start(ws[:], wqT_t[e][:, dg * 512:(dg + 1) * 512])
                    for dt in range(4):
                        nc.tensor.matmul(
                            q_psl[dt][:], ws[:, dt * 128:(dt + 1) * 128],
                            x_sb[:, e, 0:T], start=(e == 0), stop=(e == ET - 1))
                for dt in range(4):
                    st = phA3.tile([P, 512], f32r, tag="qstage",
                                   name=f"qst{dg}_{dt}")
                    nc.vector.tensor_mul(st[:], q_psl[dt][:], inv_b[:, 0:T])
                    nc.sync.dma_start(qT_d_t[dg * 4 + dt], st[:])

            # K^T [KV, S]: 2 groups of (2 d-tiles x 4 col-chunks)
            for dtg in range(2):
                k_ps = [psum.tile([P, 512], f32, tag="ps", name=f"k_ps{dtg}_{i}")
                        for i in range(8)]
                for e in range(ET):
                    ws = phAw.tile([P, 256], f32r, tag="wkstream",
                                   name=f"wsk{dtg}_{e}")
                    nc.sync.dma_start(ws[:], wkT_t[e][:, dtg * 256:(dtg + 1) * 256])
                    for i in range(8):
                        dt, kc = divmod(i, 4)
                        nc.tensor.matmul(
                            k_ps[i][:], ws[:, dt * 128:(dt + 1) * 128],
                            x_sb[:, e, kc * 512:(kc + 1) * 512],
                            start=(e == 0), stop=(e == ET - 1))
                for i in range(8):
                    dt, kc = divmod(i, 4)
                    st = phA3.tile([P, 512], f32r, tag="qstage",
                                   name=f"kst{dtg}_{i}")
                    nc.vector.tensor_mul(st[:], k_ps[i][:],
                                         inv_b[:, kc * 512:(kc + 1) * 512])
                    nc.sync.dma_start(
                        kT_d_t[dtg * 2 + dt][:, kc * 512:(kc + 1) * 512], st[:])

            # V [S, KV] token-major: 2 groups of 8 token-tiles
            for tgg in range(2):
                v_ps = [psum.tile([P, 512], f32, tag="ps", name=f"v_ps{tgg}_{i}")
                        for i in range(8)]
                for e in range(ET):
                    ws = phAw.tile([P, 512], f32r, tag="wstream",
                                   name=f"wsv{tgg}_{e}")
                    nc.sync.dma_start(ws[:], wvT_t[e])
                    for tt in range(8):
                        tok = (tgg * 8 + tt) * 128
                        nc.tensor.matmul(
                            v_ps[tt][:], x_sb[:, e, tok:tok + 128], ws[:],
                            start=(e == 0), stop=(e == ET - 1))
                for tt in range(8):
                    ti = tgg * 8 + tt
                    st = phA3.tile([P, 512], bf16, tag="vstage",
                                   name=f"vst{tgg}_{tt}")
                    nc.scalar.activation(st[:], v_ps[tt][:], Copy,
                                         scale=inv_colT[:, ti:ti + 1])
                    nc.sync.dma_start(v_d_t[ti], st[:])

        # ---------------- Phase 2: attention ------------------------------
        attn_es = ExitStack()
        attn_pool = attn_es.enter_context(
            tc.tile_pool(name="attnp", bufs=1, side="right"))
        attnT = attn_pool.tile([P, ET, T], f32r)
        with tc.tile_pool(name="phB", bufs=1) as phB, \
             tc.tile_pool(name="phB2", bufs=2) as phB2, \
             tc.tile_pool(name="phBpt", bufs=ET + 2) as phBpt:
            q_sb = phB.tile([P, ET, T], f32r)
            m_sb = phB.tile([P, ET, T], bf16)
            for e in range(ET):
                nc.sync.dma_start(q_sb[:, e, :], qT_d_t[e])
                nc.sync.dma_start(m_sb[:, e, :], maskE_t[e])

            for g in range(NKV):
                k_g = phB2.tile([P, S], f32r, tag="kg", name=f"k_g{g}")
                nc.sync.dma_start(k_g[:], kT_d_t[g])
                v_g = phB2.tile([P, ET, HD], bf16, tag="vg", name=f"v_g{g}")
                for e in range(ET):
                    nc.sync.dma_start(v_g[:, e, :],
                                      v_d_t[e][:, g * 128:(g + 1) * 128])
                for hh in range(REP):
                    h = g * REP + hh
                    pts = []
                    for kt in range(ET):
                        s_ps = psum.tile([P, 512], f32, tag="ps",
                                         name=f"s_ps{h}_{kt}")
                        nc.tensor.matmul(
                            s_ps[:], k_g[:, kt * 128:(kt + 1) * 128],
                            q_sb[:, h, :], start=True, stop=True)
                        pt = phBpt.tile([P, T], bf16, tag="pt",
                                        name=f"pt{h}_{kt}")
                        nc.scalar.activation(pt[:], s_ps[:], Exp)
                        nc.vector.tensor_mul(pt[:], pt[:], m_sb[:, kt, :])
                        pts.append(pt)
                    sum_ps = psum.tile([1, 512], f32, tag="ps",
                                       name=f"sum_ps{h}")
                    for kt in range(ET):
                        nc.tensor.matmul(sum_ps[:], ones_bf[:], pts[kt][:],
                                         start=(kt == 0), stop=(kt == ET - 1))
                    pv_ps = psum.tile([P, 512], f32, tag="ps",
                                      name=f"pv_ps{h}")
                    for kt in range(ET):
                        nc.tensor.matmul(
                            pv_ps[:], v_g[:, kt, :],
                            pts[kt][:], start=(kt == 0), stop=(kt == ET - 1))
                    inv_sum = phB2.tile([1, 512], f32, tag="invs",
                                        name=f"inv_sum{h}")
                    nc.vector.reciprocal(inv_sum[:], sum_ps[:])
                    invS_b = phB2.tile([P, 512], f32, tag="invsb",
                                       name=f"invS_b{h}")
                    nc.gpsimd.partition_broadcast(invS_b[:], inv_sum[:])
                    nc.vector.tensor_mul(attnT[:, h, :], pv_ps[:], invS_b[:])

        # ---------------- Phase 3: o_proj + residual ----------------------
        hT_es = ExitStack()
        hT_pool = hT_es.enter_context(tc.tile_pool(name="hTp", bufs=1))
        hT = hT_pool.tile([P, ET, T], f32r)
        hT_bf = hT_pool.tile([P, ET, T], bf16)
        with tc.tile_pool(name="phC", bufs=1) as phC, \
             tc.tile_pool(name="phCw", bufs=10) as phCw:
            xo_sb = phC.tile([P, ET, T], f32r)
            for e in range(ET):
                nc.sync.dma_start(xo_sb[:, e, :], xT_t[e][:, 0:T])
            for og in range(4):
                o_ps = [psum.tile([P, 512], f32, tag="ps", name=f"o_ps{og}_{i}")
                        for i in range(4)]
                for e in range(ET):
                    ws = phCw.tile([P, 512], f32r, tag="wostream",
                                   name=f"wso{og}_{e}")
                    nc.sync.dma_start(ws[:], woT_t[e][:, og * 512:(og + 1) * 512])
                    for ot in range(4):
                        nc.tensor.matmul(
                            o_ps[ot][:], ws[:, ot * 128:(ot + 1) * 128],
                            attnT[:, e, :], start=(e == 0), stop=(e == ET - 1))
                for ot in range(4):
                    o = og * 4 + ot
                    nc.vector.tensor_tensor(
                        out=hT[:, o, :], in0=o_ps[ot][:],
                        in1=xo_sb[:, o, :], op=ADD)
                    nc.vector.tensor_copy(hT_bf[:, o, :], hT[:, o, :])
        attn_es.close()

        # ---------------- Phase 4: rmsnorm 2 ------------------------------
        with tc.tile_pool(name="phD", bufs=2) as phD:
            rms2_ps = psum.tile([1, 512], f32, tag="ps")
            for o in range(ET):
                sq = phD.tile([P, T], f32r, tag="sq2", name=f"sq2_{o}")
                nc.scalar.square(sq[:], hT[:, o, :])
                nc.tensor.matmul(rms2_ps[:], ones[:], sq[:],
                                 start=(o == 0), stop=(o == ET - 1))
            inv2_row = phD.tile([1, T], f32, tag="i2r")
            nc.scalar.activation(inv2_row[:], rms2_ps[:], Sqrt,
                                 bias=eps_t[0:1, :], scale=1.0 / H)
            nc.vector.reciprocal(inv2_row[:], inv2_row[:])
            nc.gpsimd.partition_broadcast(inv2_b[:], inv2_row[:])

        # ---------------- Phase 5: MLP (bf16) -----------------------------
        with tc.tile_pool(name="phE", bufs=1) as phE, \
             tc.tile_pool(name="phEw", bufs=10) as phEw, \
             tc.tile_pool(name="phE3", bufs=3) as phE3:
            actT = phE.tile([P, FT, T], bf16)
            gact = phE.tile([P, 8, T], bf16)
            for fg in range(8):
                g_ps = [psum.tile([P, 512], f32, tag="ps", name=f"g_ps{fg}_{i}")
                        for i in range(8)]
                for e in range(ET):
                    ws = phEw.tile([P, 1024], bf16, tag="wgstream",
                                   name=f"wsg{fg}_{e}")
                    nc.sync.dma_start(ws[:], wgT_t[e][:, fg * 1024:(fg + 1) * 1024])
                    for ft in range(8):
                        nc.tensor.matmul(
                            g_ps[ft][:], ws[:, ft * 128:(ft + 1) * 128],
                            hT_bf[:, e, :], start=(e == 0), stop=(e == ET - 1))
                for ft in range(8):
                    nc.vector.tensor_mul(gact[:, ft, :], g_ps[ft][:], inv2_b[:])
                    nc.scalar.activation(gact[:, ft, :], gact[:, ft, :], Silu)
                u_ps = [psum.tile([P, 512], f32, tag="ps", name=f"u_ps{fg}_{i}")
                        for i in range(8)]
                for e in range(ET):
                    ws = phEw.tile([P, 1024], bf16, tag="wgstream",
                                   name=f"wsu{fg}_{e}")
                    nc.sync.dma_start(ws[:], wuT_t[e][:, fg * 1024:(fg + 1) * 1024])
                    for ft in range(8):
                        nc.tensor.matmul(
                            u_ps[ft][:], ws[:, ft * 128:(ft + 1) * 128],
                            hT_bf[:, e, :], start=(e == 0), stop=(e == ET - 1))
                for ft in range(8):
                    st = phE3.tile([P, T], bf16, tag="actstage",
                                   name=f"ast{fg}_{ft}")
                    nc.vector.tensor_mul(st[:], u_ps[ft][:], inv2_b[:])
                    nc.vector.tensor_mul(actT[:, fg * 8 + ft, :], st[:],
                                         gact[:, ft, :])

            # down proj + residual
            for og in range(2):
                d_ps = [psum.tile([P, 512], f32, tag="ps", name=f"d_ps{og}_{i}")
                        for i in range(8)]
                for f in range(FT):
                    ws = phEw.tile([P, 1024], bf16, tag="wdstream",
                                   name=f"wsd{og}_{f}")
                    nc.sync.dma_start(ws[:], wdT_t[f][:, og * 1024:(og + 1) * 1024])
                    for ot in range(8):
                        nc.tensor.matmul(
                            d_ps[ot][:], ws[:, ot * 128:(ot + 1) * 128],
                            actT[:, f, :], start=(f == 0), stop=(f == FT - 1))
                for ot in range(8):
                    o = og * 8 + ot
                    st = phE3.tile([P, T], f32, tag="ostage",
                                   name=f"ost{og}_{ot}")
                    nc.vector.tensor_tensor(
                        out=st[:], in0=d_ps[ot][:], in1=hT[:, o, :], op=ADD)
                    nc.sync.dma_start(outT_t[o], st[:])
        hT_es.close()

    nc.finalize()
    return nc


_NC = None
LAST_RESULT = None


def prepare_in_maps(x, attention_mask, wq, wk, wv, wo, norm1_w, norm2_w,
                    w_gate, w_up, w_down):
    x = np.asarray(x, dtype=np.float32)
    mask = np.asarray(attention_mask, dtype=np.float32)[0, 0]  # [S, S]
    n1 = np.asarray(norm1_w, dtype=np.float32)
    n2 = np.asarray(norm2_w, dtype=np.float32)

    wqT = np.ascontiguousarray(
        (np.asarray(wq, np.float32) * (n1[None, :] / np.sqrt(HD))).T)
    wkT = np.ascontiguousarray((np.asarray(wk, np.float32) * n1[None, :]).T)
    wvT = np.ascontiguousarray((np.asarray(wv, np.float32) * n1[None, :]).T)
    woT = np.ascontiguousarray(np.asarray(wo, np.float32).T)
    wgT = np.ascontiguousarray(
        (np.asarray(w_gate, np.float32) * n2[None, :]).T).astype(ml_dtypes.bfloat16)
    wuT = np.ascontiguousarray(
        (np.asarray(w_up, np.float32) * n2[None, :]).T).astype(ml_dtypes.bfloat16)
    wdT = np.ascontiguousarray(
        np.asarray(w_down, np.float32).T).astype(ml_dtypes.bfloat16)
    ones = np.ones((P, 1), dtype=np.float32)
    ones_bf = np.ones((P, 1), dtype=ml_dtypes.bfloat16)

    in_maps = []
    for c in range(8):
        b, j = divmod(c, 4)
        xr = np.roll(x[b], -T * j, axis=0)            # rolled sequence
        xT = np.ascontiguousarray(xr.T)               # [H, S]
        mrows = mask[T * j:T * (j + 1), :]            # own query rows
        maskE = np.ascontiguousarray(
            np.exp(np.roll(mrows, -T * j, axis=1)).T).astype(ml_dtypes.bfloat16)
        in_maps.append({
            "xT": xT, "maskE": maskE, "wqT": wqT, "wkT": wkT, "wvT": wvT,
            "woT": woT, "wgT": wgT, "wuT": wuT, "wdT": wdT, "ones_d": ones,
            "onesb_d": ones_bf,
        })
    return in_maps


def kernel(x, attention_mask, wq, wk, wv, wo, norm1_w, norm2_w,
           w_gate, w_up, w_down):
    global _NC, LAST_RESULT
    if _NC is None:
        _NC = _build()
    nc = _NC

    in_maps = prepare_in_maps(x, attention_mask, wq, wk, wv, wo, norm1_w,
                              norm2_w, w_gate, w_up, w_down)

    res = run_bass_kernel_spmd(nc, in_maps, core_ids=list(range(8)))
    LAST_RESULT = res

    out = np.empty((2, S, H), dtype=np.float32)
    for c in range(8):
        b, j = divmod(c, 4)
        out[b, T * j:T * (j + 1), :] = res.results[c]["outT"].T
    return out
